# revision 17
# baseline (speedup 1.0000x reference)
"""NF4-quantized linear + LoRA kernel for Trainium2 (Bass/Tile), 8-core SPMD.

Contract: kernel(**inputs) takes the FULL unsharded inputs
    x      [4096, 4096] float32
    codes  [4096, 4096] int32   (NF4 code indices, 0..15)
    scales [262144]     float32 (one absmax scale per 64 contiguous elements)
    lora_A [16, 4096]   float32
    lora_B [4096, 16]   float32
and returns the full output  y = x @ dequant(codes, scales).T + (x @ A.T) @ B.T * 2.0
of shape [4096, 4096] float32.

Sharding: tensor-parallel over out_features (column parallel). Each of the 8
NeuronCores gets codes/scales/lora_B rows for its 512 output columns plus a
full replica of x and lora_A, computes y_shard [4096, 512] on device, and the
shards are concatenated on the host.

Device algorithm per core (v2):
  1. Dequantize W on-chip with a degree-7 polynomial in u=(c-7.5)/7.5 fitted
     to the NF4 codebook (0.26% rel rms, far under the 2e-2 budget): ScalarE
     casts int32 codes to fp16 u (affine fused into the cast), then a Horner
     chain of scalar_tensor_tensor ops on VectorE/GpSimd, with the block
     scales applied by the final fused op via a broadcast-expanded tile.
  2. The LoRA correction W' = W + 2*(B @ A) is folded into the W transpose:
     each [i,o] PSUM chunk accumulates transpose(W chunk) + A^T chunk @ (2B^T)
     in one PSUM group, drained once to the resident fp16 W^T.
  3. x streams as fp32, is PE-transposed in fp32 (no separate cast pass), and
     the PSUM drain converts to fp16 (ScalarE/VectorE alternating).
  4. Per 128-token tile: 32 accumulating K=128 fp16 matmuls into PSUM
     [128,512] fp32, drained by ScalarE, DMA'd to HBM.
  Emission is interleaved so the x pipeline and matmuls overlap the dequant.
"""
import numpy as np

import concourse.bass as bass
import concourse.bacc as bacc
import concourse.mybir as mybir
import concourse.tile as tile
from concourse.masks import make_identity

dt = mybir.dt
A_ = mybir.AluOpType

NF4 = np.array([-1.0, -0.6961928009986877, -0.5250730514526367, -0.39491748809814453,
                -0.28444138169288635, -0.18477343022823334, -0.09105003625154495, 0.0,
                0.07958029955625534, 0.16093020141124725, 0.24611230194568634,
                0.33791524171829224, 0.44070982933044434, 0.5626170039176941,
                0.7229568362236023, 1.0], dtype=np.float64)

# deg-7 fit of NF4[c] against u = (c - 7.5) / 7.5;  POLY[0] = a7 ... POLY[7] = a0
POLY = np.polyfit((np.arange(16) - 7.5) / 7.5, NF4, 7)

N_CORES = 8
T, IN, OUT, R = 4096, 4096, 4096, 16
O = OUT // N_CORES          # 512 out columns per core
BLK = 64                    # quant block size
SCALING = 2.0               # lora_alpha / r
N_OT = O // 128             # o-tiles per core (4)
N_IC = IN // 128            # contraction chunks (32)
SUB = 512                   # dequant sub-tile width (DVE drain sweet spot)
N_SUB = IN // SUB           # subtiles per o-tile (8)
N_TT = T // 128             # token tiles (32)
XLEAD = 6                   # x tiles staged ahead of the matmul sweep
NPRO = 3                    # token tiles matmul'd per-o-block during the W phase


def _build(nc):
    x_d = nc.dram_tensor("x", [T, IN], dt.float32, kind="ExternalInput").ap()
    codes_d = nc.dram_tensor("codes", [O, IN], dt.int32, kind="ExternalInput").ap()
    scales_d = nc.dram_tensor("scales", [O, IN // BLK], dt.float32, kind="ExternalInput").ap()
    lora_a_d = nc.dram_tensor("lora_a", [R, IN], dt.float32, kind="ExternalInput").ap()
    lora_b_d = nc.dram_tensor("lora_b", [O, R], dt.float32, kind="ExternalInput").ap()
    y_d = nc.dram_tensor("y", [T, O], dt.float32, kind="ExternalOutput").ap()

    with tile.TileContext(nc) as tc:
        with tc.tile_pool(name="persist", bufs=1) as pp:
            wt = pp.tile([128, N_IC * O], dt.float16, name="wt")
            ident = pp.tile([128, 128], dt.float16, name="ident")
            identf = pp.tile([128, 128], dt.float32, name="identf")
            make_identity(nc, ident)
            make_identity(nc, identf)

            # ---- LoRA prep: a16 [r, i] fp16;  bt [r, o] fp16 scaled by 2.0 ----
            a16 = pp.tile([R, IN], dt.float16, name="a16")
            bt = pp.tile([R, O], dt.float16, name="bt")
            with tc.tile_pool(name="lora", bufs=1) as lp, \
                 tc.tile_pool(name="lpsum", bufs=2, space="PSUM") as lps:
                a_f = lp.tile([R, IN], dt.float32)
                nc.sync.dma_start(out=a_f, in_=lora_a_d)
                nc.scalar.copy(a16, a_f)
                b_f = lp.tile([128, N_OT * R], dt.float32)
                for b in range(N_OT):
                    nc.sync.dma_start(out=b_f[:, b * R:(b + 1) * R],
                                      in_=lora_b_d[b * 128:(b + 1) * 128, :])
                b16 = lp.tile([128, N_OT * R], dt.float16)
                nc.scalar.copy(b16, b_f)
                for b in range(N_OT):
                    pst = lps.tile([R, 128], dt.float16)
                    nc.tensor.transpose(pst, b16[:, b * R:(b + 1) * R], ident)
                    nc.scalar.activation(bt[:, b * 128:(b + 1) * 128], pst,
                                         mybir.ActivationFunctionType.Copy,
                                         scale=SCALING)

            with tc.tile_pool(name="wsc", bufs=1) as wsp, \
                 tc.tile_pool(name="cod", bufs=10) as cp, \
                 tc.tile_pool(name="deq", bufs=1) as dq, \
                 tc.tile_pool(name="wpr", bufs=2) as wp, \
                 tc.tile_pool(name="xf", bufs=6) as xp, \
                 tc.tile_pool(name="xt", bufs=XLEAD + 2) as xtp, \
                 tc.tile_pool(name="tpsum", bufs=2, space="PSUM") as tps, \
                 tc.tile_pool(name="mout", bufs=2) as mp:

                # ---- scales: [512, 64] fp32 -> scal fp16 [128, 4*64] ----
                scal = wsp.tile([128, N_OT * (IN // BLK)], dt.float16, name="scal")
                scal_f = wsp.tile([128, N_OT * (IN // BLK)], dt.float32, name="scal_f")
                for b in range(N_OT):
                    nc.sync.dma_start(out=scal_f[:, b * 64:(b + 1) * 64],
                                      in_=scales_d[b * 128:(b + 1) * 128, :])
                nc.scalar.copy(scal, scal_f)

                # ---------- W phase pieces ----------
                def w_codes_dma(b):
                    chunks = []
                    for g in range(N_SUB):
                        cb = cp.tile([128, SUB], dt.int32, tag="codes")
                        nc.sync.dma_start(
                            out=cb, in_=codes_d[b * 128:(b + 1) * 128,
                                                g * SUB:(g + 1) * SUB])
                        chunks.append(cb)
                    return chunks

                def w_dequant_sub(b, g, cb, wpr):
                    """Horner chain for subtile g of o-tile b into wpr[:, g*SUB:]."""
                    eng = nc.vector
                    par = g % 2
                    u = dq.tile([128, SUB], dt.bfloat16, tag=f"u{par}")
                    # u = (c - 7.5) / 7.5, fused into the int->fp16 cast
                    nc.scalar.activation(u, cb, mybir.ActivationFunctionType.Copy,
                                         bias=-1.0, scale=1.0 / 7.5)
                    # sexp: block scales broadcast-expanded 64x
                    sexp = dq.tile([128, SUB], dt.bfloat16, tag=f"sx{par}")
                    sc = scal[:, b * 64 + g * (SUB // BLK): b * 64 + (g + 1) * (SUB // BLK)]
                    s_b = bass.AP(sc.tensor, sc.offset, [sc.ap[0], sc.ap[1], [0, BLK]])
                    nc.scalar.copy(sexp.rearrange("p (k j) -> p k j", j=BLK), s_b)
                    p = dq.tile([128, SUB], dt.bfloat16, tag=f"p{par}")
                    q = dq.tile([128, SUB], dt.bfloat16, tag=f"q{par}")
                    eng.tensor_scalar(p, u, float(POLY[0]), None, op0=A_.mult)
                    cur, nxt = p, q
                    for k in range(1, 7):
                        eng.scalar_tensor_tensor(nxt, cur, float(POLY[k]), u,
                                                 op0=A_.add, op1=A_.mult)
                        cur, nxt = nxt, cur
                    # w = (p + a0) * scale  (fp32 out so the PE can transpose+
                    # accumulate the LoRA matmul in one fp32 PSUM group)
                    eng.scalar_tensor_tensor(wpr[:, g * SUB:(g + 1) * SUB], cur,
                                             float(POLY[7]), sexp,
                                             op0=A_.add, op1=A_.mult)

                def w_fold_transpose(b, wpr):
                    """PE: transpose wpr chunks + accumulate 2*(BA)^T; drain to wt."""
                    for grp in range(N_IC // 4):
                        tp = tps.tile([128, 4 * 128], dt.float32, tag="wtp", bufs=2)
                        for k in range(4):
                            c = grp * 4 + k
                            sl = slice(k * 128, (k + 1) * 128)
                            nc.tensor.matmul(tp[:, sl], wpr[:, c * 128:(c + 1) * 128],
                                             identf, is_transpose=True,
                                             start=True, stop=False)
                            # += A^T chunk @ (2 B^T) block:  [i,o] low-rank update
                            nc.tensor.matmul(tp[:, sl],
                                             a16[:, c * 128:(c + 1) * 128],
                                             bt[:, b * 128:(b + 1) * 128],
                                             start=False, stop=True)
                        outap = bass.AP(wt.tensor, wt.offset + grp * 4 * O + b * 128,
                                        [wt.ap[0], [O, 4], [1, 128]])
                        nc.scalar.copy(outap, tp.rearrange("p (k f) -> p k f", k=4))

                # ---------- x phase pieces ----------
                def x_stage(it, w_phase=False):
                    xt = xtp.tile([128, IN], dt.float16, tag="xt")
                    for k in range(4):
                        xf = xp.tile([128, 1024], dt.float32, tag="xf")
                        nc.sync.dma_start(
                            out=xf, in_=x_d[it * 128:(it + 1) * 128,
                                            k * 1024:(k + 1) * 1024])
                        for h in range(2):
                            tp = tps.tile([128, 512], dt.float32, tag="tp")
                            for j in range(4):
                                nc.tensor.transpose(
                                    tp[:, j * 128:(j + 1) * 128],
                                    xf[:, (h * 4 + j) * 128:(h * 4 + j + 1) * 128],
                                    identf)
                            # during the W phase VectorE is busy with dequant
                            dr = nc.scalar.copy if (w_phase or (2 * k + h) % 2 == 0) \
                                else nc.vector.tensor_copy
                            dr(xt[:, (k * 2 + h) * 512:(k * 2 + h + 1) * 512], tp)
                    return xt

                def drain_y(it, yps):
                    yo = mp.tile([128, O], dt.float32, tag="yo")
                    nc.scalar.copy(yo, yps)
                    nc.sync.dma_start(out=y_d[it * 128:(it + 1) * 128, :], in_=yo)

                # ---------- interleaved emission ----------
                # Codes DMA for all o-tiles first so dequant is never DMA-starved.
                code_chunks = [w_codes_dma(b) for b in range(N_OT)]

                xts = {}
                # W phase + prologue: tiles 0..NPRO-1 accumulate per-o-block as
                # each W block lands; their PSUM tiles live in a pool that is
                # closed (banks freed) before the steady-state loop.
                with tc.tile_pool(name="propsum", bufs=1, space="PSUM") as prp:
                    pyps = []
                    for t in range(NPRO):
                        pyp = prp.tile([128, O], dt.float32, name=f"pyp{t}")
                        pyps.append(pyp)
                    xts[0] = x_stage(0, w_phase=True)
                    xts[1] = x_stage(1, w_phase=True)
                    nxt_stage = 2
                    for b in range(N_OT):
                        wpr = wp.tile([128, IN], dt.float32, tag="wpr")
                        for g in range(N_SUB):
                            w_dequant_sub(b, g, code_chunks[b][g], wpr)
                        # keep PE warm between W transposes
                        if nxt_stage < XLEAD:
                            xts[nxt_stage] = x_stage(nxt_stage, w_phase=True)
                            nxt_stage += 1
                        w_fold_transpose(b, wpr)
                        bsl = slice(b * 128, (b + 1) * 128)
                        for t in range(NPRO):
                            for c in range(N_IC):
                                nc.tensor.matmul(
                                    pyps[t][:, bsl],
                                    xts[t][:, c * 128:(c + 1) * 128],
                                    wt[:, c * O + b * 128:c * O + (b + 1) * 128],
                                    start=(c == 0), stop=(c == N_IC - 1))
                    while nxt_stage < XLEAD:
                        xts[nxt_stage] = x_stage(nxt_stage, w_phase=True)
                        nxt_stage += 1
                    for t in range(NPRO):
                        drain_y(t, pyps[t])
                        xts.pop(t)

                with tc.tile_pool(name="mpsum2", bufs=2, space="PSUM") as mps2:
                    for it in range(NPRO, N_TT):
                        nst = it + XLEAD - NPRO
                        if nst < N_TT:
                            xts[nst] = x_stage(nst)
                        yps = mps2.tile([128, O], dt.float32, tag="ypsum")
                        for c in range(N_IC):
                            nc.tensor.matmul(yps, xts[it][:, c * 128:(c + 1) * 128],
                                             wt[:, c * O:(c + 1) * O],
                                             start=(c == 0), stop=(c == N_IC - 1))
                        xts.pop(it)
                        drain_y(it, yps)
    return nc


_CACHE = {}


def _get_runner():
    if "r" in _CACHE:
        return _CACHE["r"]
    nc = bacc.Bacc("TRN2", target_bir_lowering=False, debug=False)
    _build(nc)
    nc.compile()

    import jax
    from jax.experimental.shard_map import shard_map
    from jax.sharding import Mesh, PartitionSpec, NamedSharding
    from concourse.bass2jax import _bass_exec_p, partition_id_tensor, install_neuronx_cc_hook

    install_neuronx_cc_hook()
    in_names, out_names, out_avals = [], [], []
    partition_name = nc.partition_id_tensor.name if nc.partition_id_tensor else None
    for alloc in nc.m.functions[0].allocations:
        if not isinstance(alloc, mybir.MemoryLocationSet):
            continue
        name = alloc.memorylocations[0].name
        if alloc.kind == "ExternalInput":
            if name != partition_name:
                in_names.append(name)
        elif alloc.kind == "ExternalOutput":
            out_names.append(name)
            out_avals.append(jax.core.ShapedArray(tuple(alloc.tensor_shape),
                                                  mybir.dt.np(alloc.dtype)))
    n_params = len(in_names)
    all_in_names = list(in_names) + list(out_names)
    if partition_name is not None:
        all_in_names.append(partition_name)

    def _body(*args):
        operands = list(args)
        if partition_name is not None:
            operands.append(partition_id_tensor())
        return tuple(_bass_exec_p.bind(
            *operands,
            out_avals=tuple(out_avals),
            in_names=tuple(all_in_names),
            out_names=tuple(out_names),
            lowering_input_output_aliases=(),
            sim_require_finite=True,
            sim_require_nnan=True,
            nc=nc,
        ))

    devices = jax.devices()[:N_CORES]
    mesh = Mesh(np.asarray(devices), ("core",))
    n_outs = len(out_avals)
    fn = jax.jit(
        shard_map(_body, mesh=mesh,
                  in_specs=(PartitionSpec("core"),) * (n_params + n_outs),
                  out_specs=(PartitionSpec("core"),) * n_outs,
                  check_rep=False),
        donate_argnums=tuple(range(n_params, n_params + n_outs)),
        keep_unused=True)
    sharding = NamedSharding(mesh, PartitionSpec("core"))
    _CACHE["r"] = (fn, in_names, out_names, out_avals, sharding)
    return _CACHE["r"]


def kernel(x, codes, scales, lora_A, lora_B):
    import jax
    fn, in_names, out_names, out_avals, sharding = _get_runner()

    x = np.ascontiguousarray(x, dtype=np.float32)
    codes = np.ascontiguousarray(codes, dtype=np.int32)
    scales2 = np.ascontiguousarray(scales, dtype=np.float32).reshape(OUT, IN // BLK)
    lora_A = np.ascontiguousarray(lora_A, dtype=np.float32)
    lora_B = np.ascontiguousarray(lora_B, dtype=np.float32)

    per_core = {
        "x": [x] * N_CORES,
        "codes": [codes[c * O:(c + 1) * O] for c in range(N_CORES)],
        "scales": [scales2[c * O:(c + 1) * O] for c in range(N_CORES)],
        "lora_a": [lora_A] * N_CORES,
        "lora_b": [lora_B[c * O:(c + 1) * O] for c in range(N_CORES)],
    }
    concat_in = [np.concatenate(per_core[n], axis=0) for n in in_names]
    dev_in = [jax.device_put(a, sharding) for a in concat_in]
    zeros = [jax.device_put(
        np.zeros((N_CORES * av.shape[0], *av.shape[1:]), av.dtype), sharding)
        for av in out_avals]
    outs = fn(*dev_in, *zeros)
    y_all = np.asarray(outs[out_names.index("y")])  # [8*4096, 512]
    y_shards = y_all.reshape(N_CORES, T, O)
    return np.concatenate([y_shards[c] for c in range(N_CORES)], axis=1)
